# revision 7
# baseline (speedup 1.0000x reference)
"""Trainium2 Bass kernel for nn_ConnectLoss (pairwise BCE-Dice instance loss).

Strategy (8 NeuronCores, pixel-sharded):
  - Each core gets H/8 = 256 rows (524288 pixels) of all four inputs.
  - Inputs are cast to fp16 on the host (plus a host-computed 1-cls plane so
    ln(1-cls) never sees a catastrophically rounded 1.0) so all DMAs ride
    HWDGE at half the bytes.
  - Heavy part is the joint histogram inter[N=16, K=32] between target/pred
    instance labels. Per core: build fp16 indicator planes on DVE and
    contract 128 pixels/matmul on TensorE into PSUM. Pred classes are
    DUAL-PACKED two per plane as [pm==2e] + 8192*[pm==2e+1] (exact in fp32
    PSUM, decoded per-core on the host), which halves the scattered
    moving-operand reads per matmul - the SBUF line-touch rate on the PE
    moving port is the real limit. The 4096 group-matmuls are round-robined
    over two 128x32 column tiles of the PE array, each accumulating into its
    own PSUM bank.
  - Moving operand also carries (cls, ln(cls), ln(1-cls)) planes so the same
    matmuls yield the per-target-class sums needed for the cls_out BCE term.
  - sum(pred_score^2) via ACT Square with accum_out.
  - Marginals st/sp from inter row/col sums; tiny final math on host.
"""

import sys

if "/opt/trn_rl_repo" not in sys.path:
    sys.path.insert(0, "/opt/trn_rl_repo")

import numpy as np
from contextlib import ExitStack

# ---------------------------------------------------------------- constants
P = 128
H, W = 2048, 2048
NCORES = 8
ROWS = H // NCORES                 # 256 rows per core
PIX = ROWS * W                     # 524288 pixels per core
FPP = PIX // P                     # 4096 free elems per partition
CHUNKS = [256, 1024, 1024, 1024, 512, 256]   # small head/tail chunks
assert sum(CHUNKS) == FPP
NCHUNK = len(CHUNKS)
K = 32                             # pred instance classes
ND = K // 2                        # 16 dual-packed pred planes
N = 16                             # target instance classes
NQUAD = 2                          # PE column tiles in use (128x32 mode)
SCALE = 8192.0                     # dual-pack base (counts < 8192 per shard)
MCOL = ND + 3                      # 19 moving cols: 16 duals + cls/ln/ln1m
OUTC = 48                          # out cols: [0:19] inter+aux, [40:45] ps2

SMOOTH = 1.0
HWPIX = float(H * W)

_cached = {}


def _build_bass():
    import concourse.bass as bass
    import concourse.bacc as bacc
    import concourse.mybir as mybir
    from concourse.tile import TileContext

    f32 = mybir.dt.float32
    f16 = mybir.dt.float16
    eq = mybir.AluOpType.is_equal
    mult = mybir.AluOpType.mult
    add = mybir.AluOpType.add
    AF = mybir.ActivationFunctionType

    nc = bacc.Bacc("TRN2")
    pm_d = nc.dram_tensor("pm", [PIX], f16, kind="ExternalInput")
    tm_d = nc.dram_tensor("tm", [PIX], f16, kind="ExternalInput")
    cls_d = nc.dram_tensor("cls", [PIX], f16, kind="ExternalInput")
    omc_d = nc.dram_tensor("omc", [PIX], f16, kind="ExternalInput")
    ps_d = nc.dram_tensor("ps", [PIX], f16, kind="ExternalInput")
    out_d = nc.dram_tensor("out", [P, OUTC], f32, kind="ExternalOutput")

    pm_v = pm_d[:].rearrange("(p f) -> p f", p=P)
    tm_v = tm_d[:].rearrange("(p f) -> p f", p=P)
    cls_v = cls_d[:].rearrange("(p f) -> p f", p=P)
    omc_v = omc_d[:].rearrange("(p f) -> p f", p=P)
    ps_v = ps_d[:].rearrange("(p f) -> p f", p=P)

    with ExitStack() as es:
        tc = es.enter_context(TileContext(nc))
        pool_in = es.enter_context(tc.tile_pool(name="inp", bufs=2))
        pool_toh = es.enter_context(tc.tile_pool(name="toh", bufs=2))
        pool_poh = es.enter_context(tc.tile_pool(name="poh", bufs=2))
        pool_misc = es.enter_context(tc.tile_pool(name="misc", bufs=1))
        pool_scr = es.enter_context(tc.tile_pool(name="scr", bufs=2))
        psum = es.enter_context(tc.tile_pool(name="ps", bufs=1, space="PSUM"))

        inter_ps = [
            psum.tile([P, MCOL], f32, tag=f"q{q}", name=f"interps{q}")
            for q in range(NQUAD)
        ]
        ps2acc = pool_misc.tile([P, NCHUNK], f32)
        inter_sb = pool_misc.tile([P, MCOL], f32)
        nc.gpsimd.memset(inter_sb[:], 0.0)

        off = 0
        for c, CFc in enumerate(CHUNKS):
            cs = slice(off, off + CFc)
            pm16 = pool_in.tile([P, CFc], f16, tag="pm16")
            tm16 = pool_in.tile([P, CFc], f16, tag="tm16")
            ps_t = pool_in.tile([P, CFc], f16, tag="ps")
            omc_t = pool_in.tile([P, CFc], f16, tag="omc")
            pohv = pool_poh.tile([P, MCOL * CFc], f16, tag="pohv")
            nc.sync.dma_start(out=pm16[:], in_=pm_v[:, cs])
            nc.sync.dma_start(out=tm16[:], in_=tm_v[:, cs])
            # cls lands directly in its moving-plane slot
            nc.scalar.dma_start(
                out=pohv[:, ND * CFc:(ND + 1) * CFc], in_=cls_v[:, cs])
            nc.scalar.dma_start(out=omc_t[:], in_=omc_v[:, cs])
            nc.scalar.dma_start(out=ps_t[:], in_=ps_v[:, cs])

            # dual-packed pred planes: [pm==2e] + 8192*[pm==2e+1]
            # (built with tensor_scalar 4x + tensor_tensor 2x; the fused
            # scalar_tensor_tensor only has a 1x uop and is slower)
            for e in range(ND):
                scrb = pool_scr.tile([P, CFc], f16, tag="scrb")
                nc.vector.tensor_scalar(
                    pohv[:, e * CFc:(e + 1) * CFc],
                    pm16[:], float(2 * e), None, eq,
                )
                nc.vector.tensor_scalar(
                    scrb[:], pm16[:], float(2 * e + 1), SCALE, eq, mult
                )
                # fold the 8192-scaled plane in via the SDMA CCE adder
                # (SBUF->SBUF accumulate DMA; keeps the add off the DVE)
                nc.gpsimd.dma_start(
                    out=pohv[:, e * CFc:(e + 1) * CFc], in_=scrb[:],
                    accum_op=add,
                )
            # ln(cls), ln(1-cls) on ACT
            nc.scalar.activation(
                pohv[:, (ND + 1) * CFc:(ND + 2) * CFc],
                pohv[:, ND * CFc:(ND + 1) * CFc], AF.Ln,
            )
            nc.scalar.activation(
                pohv[:, (ND + 2) * CFc:(ND + 3) * CFc],
                omc_t[:], AF.Ln,
            )
            # sum(pred_score^2) per partition for this chunk
            scr = pool_scr.tile([P, CFc], f16, tag="scr")
            nc.scalar.activation(
                scr[:], ps_t[:], AF.Square,
                accum_out=ps2acc[:, c:c + 1],
            )

            # target one-hots, whole chunk, class-major
            toh = pool_toh.tile([P, N * CFc], f16, tag="toh")
            for n in range(N):
                nc.vector.tensor_scalar(
                    toh[:, n * CFc:(n + 1) * CFc], tm16[:], float(n), None, eq
                )

            poh3 = pohv[:].rearrange("p (m f) -> p f m", m=MCOL)
            toh3 = toh[:].rearrange("p (n f) -> p f n", n=N)
            for j in range(CFc):
                q = j % NQUAD
                first = (c == 0 and j < NQUAD)
                last = (c == NCHUNK - 1 and j >= CFc - NQUAD)
                nc.tensor.matmul(
                    inter_ps[q][32 * q:32 * q + N, :],
                    toh3[:, j:j + 1, :],
                    poh3[:, j:j + 1, :],
                    start=first,
                    stop=last,
                    tile_position=(0, 32 * q),
                )
            off += CFc

        for q in range(NQUAD):
            nc.scalar.copy(
                inter_sb[32 * q:32 * q + N, :],
                inter_ps[q][32 * q:32 * q + N, :],
            )
        nc.scalar.dma_start(out=out_d[:, 0:MCOL], in_=inter_sb[:])
        nc.scalar.dma_start(out=out_d[:, 40:40 + NCHUNK], in_=ps2acc[:])

    nc.finalize()
    return nc


def _get_nc():
    if "nc" not in _cached:
        _cached["nc"] = _build_bass()
    return _cached["nc"]


def make_in_maps(pred_instance_mask, pred_score, cls_out, target_mask):
    """Shard + host-cast the full inputs into per-core fp16 input maps."""
    in_maps = []
    for c in range(NCORES):
        rs = slice(c * ROWS, (c + 1) * ROWS)
        cls = np.ascontiguousarray(cls_out[rs]).reshape(-1)
        in_maps.append({
            "pm": np.ascontiguousarray(
                pred_instance_mask[rs]).reshape(-1).astype(np.float16),
            "tm": np.ascontiguousarray(
                target_mask[rs]).reshape(-1).astype(np.float16),
            "cls": cls.astype(np.float16),
            "omc": (1.0 - cls.astype(np.float64)).astype(np.float16),
            "ps": np.ascontiguousarray(
                pred_score[rs]).reshape(-1).astype(np.float16),
        })
    return in_maps


def _get_runner():
    """Build the sharded jitted executable ONCE; reuse across calls."""
    if "runner" in _cached:
        return _cached["runner"]

    import jax
    import concourse.mybir as mybir
    from jax.sharding import Mesh, PartitionSpec
    from jax.experimental.shard_map import shard_map
    from concourse import bass2jax

    bass2jax.install_neuronx_cc_hook()
    nc = _get_nc()
    partition_name = (
        nc.partition_id_tensor.name if nc.partition_id_tensor else None
    )

    in_names, out_names, out_avals, zero_outs = [], [], [], []
    for alloc in nc.m.functions[0].allocations:
        if not isinstance(alloc, mybir.MemoryLocationSet):
            continue
        name = alloc.memorylocations[0].name
        if alloc.kind == "ExternalInput":
            if name != partition_name:
                in_names.append(name)
        elif alloc.kind == "ExternalOutput":
            out_names.append(name)
            shape = tuple(alloc.tensor_shape)
            dtype = mybir.dt.np(alloc.dtype)
            out_avals.append(jax.core.ShapedArray(shape, dtype))
            zero_outs.append(np.zeros(shape, dtype))
    n_params = len(in_names)
    n_outs = len(out_avals)
    all_in_names = list(in_names) + list(out_names)
    if partition_name is not None:
        all_in_names.append(partition_name)
    donate = tuple(range(n_params, n_params + n_outs))

    def _body(*args):
        operands = list(args)
        if partition_name is not None:
            operands.append(bass2jax.partition_id_tensor())
        outs = bass2jax._bass_exec_p.bind(
            *operands,
            out_avals=tuple(out_avals),
            in_names=tuple(all_in_names),
            out_names=tuple(out_names),
            lowering_input_output_aliases=(),
            sim_require_finite=False,
            sim_require_nnan=False,
            nc=nc,
        )
        return tuple(outs)

    devices = jax.devices()[:NCORES]
    mesh = Mesh(np.asarray(devices), ("core",))
    in_specs = (PartitionSpec("core"),) * (n_params + n_outs)
    out_specs = (PartitionSpec("core"),) * n_outs
    sharded = jax.jit(
        shard_map(
            _body, mesh=mesh, in_specs=in_specs, out_specs=out_specs,
            check_rep=False,
        ),
        donate_argnums=donate,
        keep_unused=True,
    )

    def run(in_maps):
        concat_in = [
            np.concatenate([np.asarray(m[name]) for m in in_maps], axis=0)
            for name in in_names
        ]
        concat_zeros = [
            np.zeros((NCORES * z.shape[0], *z.shape[1:]), z.dtype)
            for z in zero_outs
        ]
        out_arrs = sharded(*concat_in, *concat_zeros)
        return [
            {
                name: np.asarray(out_arrs[i]).reshape(
                    NCORES, *out_avals[i].shape)[c]
                for i, name in enumerate(out_names)
            }
            for c in range(NCORES)
        ]

    def bench(in_maps, iters=20):
        """Time the sharded call with device-resident inputs."""
        import time
        from jax.sharding import NamedSharding

        concat_in = [
            np.concatenate([np.asarray(m[name]) for m in in_maps], axis=0)
            for name in in_names
        ]
        shard = NamedSharding(mesh, PartitionSpec("core"))
        dev_in = [jax.device_put(x, shard) for x in concat_in]
        zeros = [
            np.zeros((NCORES * z.shape[0], *z.shape[1:]), z.dtype)
            for z in zero_outs
        ]

        def call():
            zs = [jax.device_put(z, shard) for z in zeros]
            outs = sharded(*dev_in, *zs)
            for o in outs:
                o.block_until_ready()

        call()
        ts = []
        for _ in range(iters):
            t0 = time.perf_counter()
            call()
            ts.append(time.perf_counter() - t0)
        return min(ts), sum(ts) / len(ts)

    run.bench = bench
    _cached["runner"] = run
    return run


def kernel(pred_instance_mask, pred_score, cls_out, target_mask):
    run = _get_runner()
    in_maps = make_in_maps(pred_instance_mask, pred_score, cls_out, target_mask)
    outs = [r["out"] for r in run(in_maps)]

    acc = np.zeros((N, K), dtype=np.float64)
    aux = np.zeros((N, 3), dtype=np.float64)
    ps2 = 0.0
    for o in outs:
        o = o.astype(np.float64)
        for q in range(NQUAD):
            blk = o[32 * q:32 * q + N, 0:MCOL]
            duals = blk[:, 0:ND]
            hi = np.floor((duals + 0.5) / SCALE)
            lo = duals - hi * SCALE
            acc[:, 0::2] += lo
            acc[:, 1::2] += hi
            aux += blk[:, ND:ND + 3]
        ps2 += o[:, 40:40 + NCHUNK].sum()

    return _host_finish(acc, aux, ps2)


def _host_finish(inter, aux, ps2):
    st = inter.sum(axis=1)            # [N] target marginals
    sp = inter.sum(axis=0)            # [K] pred marginals
    sum_t = HWPIX - st[0]             # count(target > 0)
    sum_p = aux[:, 0].sum()           # sum(cls_out)
    sum_logp = aux[:, 1].sum()
    inter_cls = sum_p - aux[0, 0]     # sum over target>0 of cls_out
    bce_sum = (sum_logp - aux[0, 1]) + aux[0, 2]

    mse = ps2 / HWPIX
    bce_cls = -bce_sum / HWPIX
    dice_cls = 1.0 - (2.0 * inter_cls + SMOOTH) / (sum_p + sum_t + SMOOTH)

    union = st[:, None] + sp[None, :]
    bce_pair = 100.0 * (union - 2.0 * inter) / HWPIX
    dice_pair = 1.0 - (2.0 * inter + SMOOTH) / (union + SMOOTH)
    pair = bce_pair + dice_pair
    res = mse + bce_cls + dice_cls + pair.min(axis=1).sum()
    return np.float32(res / float(N))


# revision 9
# speedup vs baseline: 1.8455x; 1.8455x over previous
"""Trainium2 Bass kernel for nn_ConnectLoss (pairwise BCE-Dice instance loss).

Strategy (8 NeuronCores, pixel-sharded):
  - Each core gets H/8 = 256 rows (524288 pixels) of all four inputs.
  - Inputs are cast to fp16 on the host (plus a host-computed 1-cls plane so
    ln(1-cls) never sees a catastrophically rounded 1.0) so all DMAs ride
    HWDGE at half the bytes.
  - Heavy part is the joint histogram inter[N=16, K=32] between target/pred
    instance labels. Per core: build fp16 indicator planes on DVE and
    contract 128 pixels/matmul on TensorE into PSUM. Pred classes are
    DUAL-PACKED two per plane as [pm==2e] + 8192*[pm==2e+1] (exact in fp32
    PSUM, decoded per-core on the host), which halves the scattered
    moving-operand reads per matmul - the SBUF line-touch rate on the PE
    moving port is the real limit. The 4096 group-matmuls are round-robined
    over two 128x32 column tiles of the PE array, each accumulating into its
    own PSUM bank.
  - Moving operand also carries (cls, ln(cls), ln(1-cls)) planes so the same
    matmuls yield the per-target-class sums needed for the cls_out BCE term.
  - sum(pred_score^2) via ACT Square with accum_out.
  - Marginals st/sp from inter row/col sums; tiny final math on host.
"""

import sys

if "/opt/trn_rl_repo" not in sys.path:
    sys.path.insert(0, "/opt/trn_rl_repo")

import numpy as np
from contextlib import ExitStack

# ---------------------------------------------------------------- constants
P = 128
H, W = 2048, 2048
NCORES = 8
ROWS = H // NCORES                 # 256 rows per core
PIX = ROWS * W                     # 524288 pixels per core
FPP = PIX // P                     # 4096 free elems per partition
CHUNKS = [256, 1024, 1024, 1024, 512, 256]   # small head/tail chunks
assert sum(CHUNKS) == FPP
NCHUNK = len(CHUNKS)
K = 32                             # pred instance classes
NPL = 8                            # plain-packed pred classes (0..7)
ND = (K - NPL) // 2                # 12 dual-packed pred plane pairs
N = 16                             # target instance classes
NQUAD = 2                          # PE column tiles in use (128x32 mode)
SCALE = 8192.0                     # dual-pack base (counts < 8192 per shard)
MCOL = NPL + ND + 3                # 23 moving cols: 8 plain + 12 duals + aux
OUTC = 48                          # out cols: [0:19] inter+aux, [40:45] ps2

SMOOTH = 1.0
HWPIX = float(H * W)

_cached = {}


def _build_bass():
    import concourse.bass as bass
    import concourse.bacc as bacc
    import concourse.mybir as mybir
    from concourse.tile import TileContext

    f32 = mybir.dt.float32
    f16 = mybir.dt.float16
    eq = mybir.AluOpType.is_equal
    mult = mybir.AluOpType.mult
    add = mybir.AluOpType.add
    AF = mybir.ActivationFunctionType

    nc = bacc.Bacc("TRN2")
    pm_d = nc.dram_tensor("pm", [PIX], f16, kind="ExternalInput")
    tm_d = nc.dram_tensor("tm", [PIX], f16, kind="ExternalInput")
    cls_d = nc.dram_tensor("cls", [PIX], f16, kind="ExternalInput")
    omc_d = nc.dram_tensor("omc", [PIX], f16, kind="ExternalInput")
    ps_d = nc.dram_tensor("ps", [PIX], f16, kind="ExternalInput")
    out_d = nc.dram_tensor("out", [P, OUTC], f32, kind="ExternalOutput")

    pm_v = pm_d[:].rearrange("(p f) -> p f", p=P)
    tm_v = tm_d[:].rearrange("(p f) -> p f", p=P)
    cls_v = cls_d[:].rearrange("(p f) -> p f", p=P)
    omc_v = omc_d[:].rearrange("(p f) -> p f", p=P)
    ps_v = ps_d[:].rearrange("(p f) -> p f", p=P)

    with ExitStack() as es:
        tc = es.enter_context(TileContext(nc))
        pool_in = es.enter_context(tc.tile_pool(name="inp", bufs=2))
        pool_toh = es.enter_context(tc.tile_pool(name="toh", bufs=2))
        pool_poh = es.enter_context(tc.tile_pool(name="poh", bufs=2))
        pool_misc = es.enter_context(tc.tile_pool(name="misc", bufs=1))
        pool_scr = es.enter_context(tc.tile_pool(name="scr", bufs=2))
        psum = es.enter_context(tc.tile_pool(name="ps", bufs=1, space="PSUM"))

        inter_ps = [
            psum.tile([P, MCOL], f32, tag=f"q{q}", name=f"interps{q}")
            for q in range(NQUAD)
        ]
        ps2acc = pool_misc.tile([P, NCHUNK], f32)
        inter_sb = pool_misc.tile([P, MCOL], f32)
        nc.gpsimd.memset(inter_sb[:], 0.0)

        off = 0
        for c, CFc in enumerate(CHUNKS):
            cs = slice(off, off + CFc)
            pm16 = pool_in.tile([P, CFc], f16, tag="pm16")
            tm16 = pool_in.tile([P, CFc], f16, tag="tm16")
            ps_t = pool_in.tile([P, CFc], f16, tag="ps")
            omc_t = pool_in.tile([P, CFc], f16, tag="omc")
            pohv = pool_poh.tile([P, MCOL * CFc], f16, tag="pohv")
            nc.sync.dma_start(out=pm16[:], in_=pm_v[:, cs])
            nc.sync.dma_start(out=tm16[:], in_=tm_v[:, cs])
            # cls lands directly in its moving-plane slot
            nc.scalar.dma_start(
                out=pohv[:, (NPL + ND) * CFc:(NPL + ND + 1) * CFc], in_=cls_v[:, cs])
            nc.scalar.dma_start(out=omc_t[:], in_=omc_v[:, cs])
            nc.scalar.dma_start(out=ps_t[:], in_=ps_v[:, cs])

            # plain pred one-hot planes for classes 0..NPL-1
            for k in range(NPL):
                nc.vector.tensor_scalar(
                    pohv[:, k * CFc:(k + 1) * CFc], pm16[:], float(k), None, eq
                )
            # dual-packed planes [pm==c] + 8192*[pm==c+1] for the rest
            # (tensor_scalar 4x + tensor_tensor 2x; the fused
            # scalar_tensor_tensor only has a 1x uop and is slower)
            for e in range(ND):
                c0 = NPL + 2 * e
                scra = pool_scr.tile([P, CFc], f16, tag="scra")
                scrb = pool_scr.tile([P, CFc], f16, tag="scrb")
                nc.vector.tensor_scalar(
                    scra[:], pm16[:], float(c0), None, eq
                )
                nc.vector.tensor_scalar(
                    scrb[:], pm16[:], float(c0 + 1), SCALE, eq, mult
                )
                nc.vector.tensor_tensor(
                    pohv[:, (NPL + e) * CFc:(NPL + e + 1) * CFc],
                    scra[:], scrb[:], add,
                )
            # ln(cls), ln(1-cls) on ACT
            nc.scalar.activation(
                pohv[:, (NPL + ND + 1) * CFc:(NPL + ND + 2) * CFc],
                pohv[:, (NPL + ND) * CFc:(NPL + ND + 1) * CFc], AF.Ln,
            )
            nc.scalar.activation(
                pohv[:, (NPL + ND + 2) * CFc:(NPL + ND + 3) * CFc],
                omc_t[:], AF.Ln,
            )
            # sum(pred_score^2) per partition for this chunk
            scr = pool_scr.tile([P, CFc], f16, tag="scr")
            nc.scalar.activation(
                scr[:], ps_t[:], AF.Square,
                accum_out=ps2acc[:, c:c + 1],
            )

            # target one-hots, whole chunk, class-major
            toh = pool_toh.tile([P, N * CFc], f16, tag="toh")
            for n in range(N):
                nc.vector.tensor_scalar(
                    toh[:, n * CFc:(n + 1) * CFc], tm16[:], float(n), None, eq
                )

            poh3 = pohv[:].rearrange("p (m f) -> p f m", m=MCOL)
            toh3 = toh[:].rearrange("p (n f) -> p f n", n=N)
            for j in range(CFc):
                q = j % NQUAD
                first = (c == 0 and j < NQUAD)
                last = (c == NCHUNK - 1 and j >= CFc - NQUAD)
                nc.tensor.matmul(
                    inter_ps[q][32 * q:32 * q + N, :],
                    toh3[:, j:j + 1, :],
                    poh3[:, j:j + 1, :],
                    start=first,
                    stop=last,
                    tile_position=(0, 32 * q),
                )
            off += CFc

        for q in range(NQUAD):
            nc.scalar.copy(
                inter_sb[32 * q:32 * q + N, :],
                inter_ps[q][32 * q:32 * q + N, :],
            )
        nc.scalar.dma_start(out=out_d[:, 0:MCOL], in_=inter_sb[:])
        nc.scalar.dma_start(out=out_d[:, 40:40 + NCHUNK], in_=ps2acc[:])

    nc.finalize()
    return nc


def _get_nc():
    if "nc" not in _cached:
        _cached["nc"] = _build_bass()
    return _cached["nc"]


def make_in_maps(pred_instance_mask, pred_score, cls_out, target_mask):
    """Shard + host-cast the full inputs into per-core fp16 input maps."""
    in_maps = []
    for c in range(NCORES):
        rs = slice(c * ROWS, (c + 1) * ROWS)
        cls = np.ascontiguousarray(cls_out[rs]).reshape(-1)
        in_maps.append({
            "pm": np.ascontiguousarray(
                pred_instance_mask[rs]).reshape(-1).astype(np.float16),
            "tm": np.ascontiguousarray(
                target_mask[rs]).reshape(-1).astype(np.float16),
            "cls": cls.astype(np.float16),
            "omc": (1.0 - cls.astype(np.float64)).astype(np.float16),
            "ps": np.ascontiguousarray(
                pred_score[rs]).reshape(-1).astype(np.float16),
        })
    return in_maps


def _get_runner():
    """Build the sharded jitted executable ONCE; reuse across calls."""
    if "runner" in _cached:
        return _cached["runner"]

    import jax
    import concourse.mybir as mybir
    from jax.sharding import Mesh, PartitionSpec
    from jax.experimental.shard_map import shard_map
    from concourse import bass2jax

    bass2jax.install_neuronx_cc_hook()
    nc = _get_nc()
    partition_name = (
        nc.partition_id_tensor.name if nc.partition_id_tensor else None
    )

    in_names, out_names, out_avals, zero_outs = [], [], [], []
    for alloc in nc.m.functions[0].allocations:
        if not isinstance(alloc, mybir.MemoryLocationSet):
            continue
        name = alloc.memorylocations[0].name
        if alloc.kind == "ExternalInput":
            if name != partition_name:
                in_names.append(name)
        elif alloc.kind == "ExternalOutput":
            out_names.append(name)
            shape = tuple(alloc.tensor_shape)
            dtype = mybir.dt.np(alloc.dtype)
            out_avals.append(jax.core.ShapedArray(shape, dtype))
            zero_outs.append(np.zeros(shape, dtype))
    n_params = len(in_names)
    n_outs = len(out_avals)
    all_in_names = list(in_names) + list(out_names)
    if partition_name is not None:
        all_in_names.append(partition_name)
    donate = tuple(range(n_params, n_params + n_outs))

    def _body(*args):
        operands = list(args)
        if partition_name is not None:
            operands.append(bass2jax.partition_id_tensor())
        outs = bass2jax._bass_exec_p.bind(
            *operands,
            out_avals=tuple(out_avals),
            in_names=tuple(all_in_names),
            out_names=tuple(out_names),
            lowering_input_output_aliases=(),
            sim_require_finite=False,
            sim_require_nnan=False,
            nc=nc,
        )
        return tuple(outs)

    devices = jax.devices()[:NCORES]
    mesh = Mesh(np.asarray(devices), ("core",))
    in_specs = (PartitionSpec("core"),) * (n_params + n_outs)
    out_specs = (PartitionSpec("core"),) * n_outs
    sharded = jax.jit(
        shard_map(
            _body, mesh=mesh, in_specs=in_specs, out_specs=out_specs,
            check_rep=False,
        ),
        donate_argnums=donate,
        keep_unused=True,
    )

    def run(in_maps):
        concat_in = [
            np.concatenate([np.asarray(m[name]) for m in in_maps], axis=0)
            for name in in_names
        ]
        concat_zeros = [
            np.zeros((NCORES * z.shape[0], *z.shape[1:]), z.dtype)
            for z in zero_outs
        ]
        out_arrs = sharded(*concat_in, *concat_zeros)
        return [
            {
                name: np.asarray(out_arrs[i]).reshape(
                    NCORES, *out_avals[i].shape)[c]
                for i, name in enumerate(out_names)
            }
            for c in range(NCORES)
        ]

    def bench(in_maps, iters=20):
        """Time the sharded call with device-resident inputs."""
        import time
        from jax.sharding import NamedSharding

        concat_in = [
            np.concatenate([np.asarray(m[name]) for m in in_maps], axis=0)
            for name in in_names
        ]
        shard = NamedSharding(mesh, PartitionSpec("core"))
        dev_in = [jax.device_put(x, shard) for x in concat_in]
        zeros = [
            np.zeros((NCORES * z.shape[0], *z.shape[1:]), z.dtype)
            for z in zero_outs
        ]

        def call():
            zs = [jax.device_put(z, shard) for z in zeros]
            outs = sharded(*dev_in, *zs)
            for o in outs:
                o.block_until_ready()

        call()
        ts = []
        for _ in range(iters):
            t0 = time.perf_counter()
            call()
            ts.append(time.perf_counter() - t0)
        return min(ts), sum(ts) / len(ts)

    run.bench = bench
    _cached["runner"] = run
    return run


def kernel(pred_instance_mask, pred_score, cls_out, target_mask):
    run = _get_runner()
    in_maps = make_in_maps(pred_instance_mask, pred_score, cls_out, target_mask)
    outs = [r["out"] for r in run(in_maps)]

    acc = np.zeros((N, K), dtype=np.float64)
    aux = np.zeros((N, 3), dtype=np.float64)
    ps2 = 0.0
    for o in outs:
        o = o.astype(np.float64)
        for q in range(NQUAD):
            blk = o[32 * q:32 * q + N, 0:MCOL]
            acc[:, 0:NPL] += blk[:, 0:NPL]
            duals = blk[:, NPL:NPL + ND]
            hi = np.floor((duals + 0.5) / SCALE)
            lo = duals - hi * SCALE
            acc[:, NPL::2] += lo
            acc[:, NPL + 1::2] += hi
            aux += blk[:, NPL + ND:NPL + ND + 3]
        ps2 += o[:, 40:40 + NCHUNK].sum()

    return _host_finish(acc, aux, ps2)


def _host_finish(inter, aux, ps2):
    st = inter.sum(axis=1)            # [N] target marginals
    sp = inter.sum(axis=0)            # [K] pred marginals
    sum_t = HWPIX - st[0]             # count(target > 0)
    sum_p = aux[:, 0].sum()           # sum(cls_out)
    sum_logp = aux[:, 1].sum()
    inter_cls = sum_p - aux[0, 0]     # sum over target>0 of cls_out
    bce_sum = (sum_logp - aux[0, 1]) + aux[0, 2]

    mse = ps2 / HWPIX
    bce_cls = -bce_sum / HWPIX
    dice_cls = 1.0 - (2.0 * inter_cls + SMOOTH) / (sum_p + sum_t + SMOOTH)

    union = st[:, None] + sp[None, :]
    bce_pair = 100.0 * (union - 2.0 * inter) / HWPIX
    dice_pair = 1.0 - (2.0 * inter + SMOOTH) / (union + SMOOTH)
    pair = bce_pair + dice_pair
    res = mse + bce_cls + dice_cls + pair.min(axis=1).sum()
    return np.float32(res / float(N))


# revision 11
# speedup vs baseline: 1.9403x; 1.0514x over previous
"""Trainium2 Bass kernel for nn_ConnectLoss (pairwise BCE-Dice instance loss).

Strategy (8 NeuronCores, pixel-sharded):
  - Each core gets H/8 = 256 rows (524288 pixels) of all four inputs.
  - Inputs are cast to fp16 on the host (plus a host-computed 1-cls plane so
    ln(1-cls) never sees a catastrophically rounded 1.0) so all DMAs ride
    HWDGE at half the bytes.
  - Heavy part is the joint histogram inter[N=16, K=32] between target/pred
    instance labels. Per core: build fp16 indicator planes on DVE and
    contract 128 pixels/matmul on TensorE into PSUM. Pred classes are
    DUAL-PACKED two per plane as [pm==2e] + 8192*[pm==2e+1] (exact in fp32
    PSUM, decoded per-core on the host), which halves the scattered
    moving-operand reads per matmul - the SBUF line-touch rate on the PE
    moving port is the real limit. The 4096 group-matmuls are round-robined
    over two 128x32 column tiles of the PE array, each accumulating into its
    own PSUM bank.
  - Moving operand also carries (cls, ln(cls), ln(1-cls)) planes so the same
    matmuls yield the per-target-class sums needed for the cls_out BCE term.
  - sum(pred_score^2) via ACT Square with accum_out.
  - Marginals st/sp from inter row/col sums; tiny final math on host.
"""

import sys

if "/opt/trn_rl_repo" not in sys.path:
    sys.path.insert(0, "/opt/trn_rl_repo")

import numpy as np
from contextlib import ExitStack

# ---------------------------------------------------------------- constants
P = 128
H, W = 2048, 2048
NCORES = 8
ROWS = H // NCORES                 # 256 rows per core
PIX = ROWS * W                     # 524288 pixels per core
FPP = PIX // P                     # 4096 free elems per partition
CHUNKS = [256, 1024, 1024, 1024, 512, 256]   # small head/tail chunks
assert sum(CHUNKS) == FPP
NCHUNK = len(CHUNKS)
K = 32                             # pred instance classes
ND = K // 2                        # 16 dual-packed pred planes
N = 16                             # target instance classes
NQUAD = 2                          # PE column tiles in use (128x32 mode)
SCALE = 8192.0                     # dual-pack base (counts < 8192 per shard)
MCOL = ND + 3                      # 19 moving cols: 16 duals + cls/ln/ln1m
OUTC = 48                          # out cols: [0:19] inter+aux, [40:45] ps2

SMOOTH = 1.0
HWPIX = float(H * W)

_cached = {}


def _build_bass():
    import concourse.bass as bass
    import concourse.bacc as bacc
    import concourse.mybir as mybir
    from concourse.tile import TileContext

    f32 = mybir.dt.float32
    f16 = mybir.dt.float16
    eq = mybir.AluOpType.is_equal
    mult = mybir.AluOpType.mult
    add = mybir.AluOpType.add
    AF = mybir.ActivationFunctionType

    nc = bacc.Bacc("TRN2")
    pmh_d = nc.dram_tensor("pmh", [PIX], f16, kind="ExternalInput")
    pmw_d = nc.dram_tensor("pmw", [PIX], f16, kind="ExternalInput")
    tm_d = nc.dram_tensor("tm", [PIX], f16, kind="ExternalInput")
    cls_d = nc.dram_tensor("cls", [PIX], f16, kind="ExternalInput")
    omc_d = nc.dram_tensor("omc", [PIX], f16, kind="ExternalInput")
    ps_d = nc.dram_tensor("ps", [PIX], f16, kind="ExternalInput")
    out_d = nc.dram_tensor("out", [P, OUTC], f32, kind="ExternalOutput")

    pmh_v = pmh_d[:].rearrange("(p f) -> p f", p=P)
    pmw_v = pmw_d[:].rearrange("(p f) -> p f", p=P)
    tm_v = tm_d[:].rearrange("(p f) -> p f", p=P)
    cls_v = cls_d[:].rearrange("(p f) -> p f", p=P)
    omc_v = omc_d[:].rearrange("(p f) -> p f", p=P)
    ps_v = ps_d[:].rearrange("(p f) -> p f", p=P)

    with ExitStack() as es:
        tc = es.enter_context(TileContext(nc))
        pool_in = es.enter_context(tc.tile_pool(name="inp", bufs=2))
        pool_toh = es.enter_context(tc.tile_pool(name="toh", bufs=2))
        pool_poh = es.enter_context(tc.tile_pool(name="poh", bufs=2))
        pool_misc = es.enter_context(tc.tile_pool(name="misc", bufs=1))
        pool_scr = es.enter_context(tc.tile_pool(name="scr", bufs=2))
        psum = es.enter_context(tc.tile_pool(name="ps", bufs=1, space="PSUM"))

        inter_ps = [
            psum.tile([P, MCOL], f32, tag=f"q{q}", name=f"interps{q}")
            for q in range(NQUAD)
        ]
        ps2acc = pool_misc.tile([P, NCHUNK], f32)
        inter_sb = pool_misc.tile([P, MCOL], f32)
        nc.gpsimd.memset(inter_sb[:], 0.0)

        off = 0
        for c, CFc in enumerate(CHUNKS):
            cs = slice(off, off + CFc)
            pmh16 = pool_in.tile([P, CFc], f16, tag="pmh16")
            pmw16 = pool_in.tile([P, CFc], f16, tag="pmw16")
            tm16 = pool_in.tile([P, CFc], f16, tag="tm16")
            ps_t = pool_in.tile([P, CFc], f16, tag="ps")
            omc_t = pool_in.tile([P, CFc], f16, tag="omc")
            pohv = pool_poh.tile([P, MCOL * CFc], f16, tag="pohv")
            nc.sync.dma_start(out=pmh16[:], in_=pmh_v[:, cs])
            nc.sync.dma_start(out=pmw16[:], in_=pmw_v[:, cs])
            nc.sync.dma_start(out=tm16[:], in_=tm_v[:, cs])
            # cls lands directly in its moving-plane slot
            nc.scalar.dma_start(
                out=pohv[:, ND * CFc:(ND + 1) * CFc], in_=cls_v[:, cs])
            nc.scalar.dma_start(out=omc_t[:], in_=omc_v[:, cs])
            nc.scalar.dma_start(out=ps_t[:], in_=ps_v[:, cs])

            # dual-packed pred planes [pm==2e] + 8192*[pm==2e+1], via the
            # host-split labels: (pmh==e) * pmw with pmh=pm>>1 and
            # pmw = 1+8191*(pm&1). Two DVE ops per pair (TS 4x + TT 2x).
            for e in range(ND):
                scra = pool_scr.tile([P, CFc], f16, tag="scra")
                nc.vector.tensor_scalar(
                    scra[:], pmh16[:], float(e), None, eq
                )
                nc.vector.tensor_tensor(
                    pohv[:, e * CFc:(e + 1) * CFc], scra[:], pmw16[:], mult
                )
            # ln(cls), ln(1-cls) on ACT
            nc.scalar.activation(
                pohv[:, (ND + 1) * CFc:(ND + 2) * CFc],
                pohv[:, ND * CFc:(ND + 1) * CFc], AF.Ln,
            )
            nc.scalar.activation(
                pohv[:, (ND + 2) * CFc:(ND + 3) * CFc],
                omc_t[:], AF.Ln,
            )
            # sum(pred_score^2) per partition for this chunk
            scr = pool_scr.tile([P, CFc], f16, tag="scr")
            nc.scalar.activation(
                scr[:], ps_t[:], AF.Square,
                accum_out=ps2acc[:, c:c + 1],
            )

            # target one-hots, whole chunk, class-major
            toh = pool_toh.tile([P, N * CFc], f16, tag="toh")
            for n in range(N):
                nc.vector.tensor_scalar(
                    toh[:, n * CFc:(n + 1) * CFc], tm16[:], float(n), None, eq
                )

            poh3 = pohv[:].rearrange("p (m f) -> p f m", m=MCOL)
            toh3 = toh[:].rearrange("p (n f) -> p f n", n=N)
            for j in range(CFc):
                q = j % NQUAD
                first = (c == 0 and j < NQUAD)
                last = (c == NCHUNK - 1 and j >= CFc - NQUAD)
                nc.tensor.matmul(
                    inter_ps[q][32 * q:32 * q + N, :],
                    toh3[:, j:j + 1, :],
                    poh3[:, j:j + 1, :],
                    start=first,
                    stop=last,
                    tile_position=(0, 32 * q),
                )
            off += CFc

        for q in range(NQUAD):
            nc.scalar.copy(
                inter_sb[32 * q:32 * q + N, :],
                inter_ps[q][32 * q:32 * q + N, :],
            )
        nc.scalar.dma_start(out=out_d[:, 0:MCOL], in_=inter_sb[:])
        nc.scalar.dma_start(out=out_d[:, 40:40 + NCHUNK], in_=ps2acc[:])

    nc.finalize()
    return nc


def _get_nc():
    if "nc" not in _cached:
        _cached["nc"] = _build_bass()
    return _cached["nc"]


def make_in_maps(pred_instance_mask, pred_score, cls_out, target_mask):
    """Shard + host-cast the full inputs into per-core fp16 input maps."""
    in_maps = []
    for c in range(NCORES):
        rs = slice(c * ROWS, (c + 1) * ROWS)
        cls = np.ascontiguousarray(cls_out[rs]).reshape(-1)
        pm = np.ascontiguousarray(pred_instance_mask[rs]).reshape(-1)
        in_maps.append({
            "pmh": (pm >> 1).astype(np.float16),
            "pmw": (1 + 8191 * (pm & 1)).astype(np.float16),
            "tm": np.ascontiguousarray(
                target_mask[rs]).reshape(-1).astype(np.float16),
            "cls": cls.astype(np.float16),
            "omc": (1.0 - cls.astype(np.float64)).astype(np.float16),
            "ps": np.ascontiguousarray(
                pred_score[rs]).reshape(-1).astype(np.float16),
        })
    return in_maps


def _get_runner():
    """Build the sharded jitted executable ONCE; reuse across calls."""
    if "runner" in _cached:
        return _cached["runner"]

    import jax
    import concourse.mybir as mybir
    from jax.sharding import Mesh, PartitionSpec
    from jax.experimental.shard_map import shard_map
    from concourse import bass2jax

    bass2jax.install_neuronx_cc_hook()
    nc = _get_nc()
    partition_name = (
        nc.partition_id_tensor.name if nc.partition_id_tensor else None
    )

    in_names, out_names, out_avals, zero_outs = [], [], [], []
    for alloc in nc.m.functions[0].allocations:
        if not isinstance(alloc, mybir.MemoryLocationSet):
            continue
        name = alloc.memorylocations[0].name
        if alloc.kind == "ExternalInput":
            if name != partition_name:
                in_names.append(name)
        elif alloc.kind == "ExternalOutput":
            out_names.append(name)
            shape = tuple(alloc.tensor_shape)
            dtype = mybir.dt.np(alloc.dtype)
            out_avals.append(jax.core.ShapedArray(shape, dtype))
            zero_outs.append(np.zeros(shape, dtype))
    n_params = len(in_names)
    n_outs = len(out_avals)
    all_in_names = list(in_names) + list(out_names)
    if partition_name is not None:
        all_in_names.append(partition_name)
    donate = tuple(range(n_params, n_params + n_outs))

    def _body(*args):
        operands = list(args)
        if partition_name is not None:
            operands.append(bass2jax.partition_id_tensor())
        outs = bass2jax._bass_exec_p.bind(
            *operands,
            out_avals=tuple(out_avals),
            in_names=tuple(all_in_names),
            out_names=tuple(out_names),
            lowering_input_output_aliases=(),
            sim_require_finite=False,
            sim_require_nnan=False,
            nc=nc,
        )
        return tuple(outs)

    devices = jax.devices()[:NCORES]
    mesh = Mesh(np.asarray(devices), ("core",))
    in_specs = (PartitionSpec("core"),) * (n_params + n_outs)
    out_specs = (PartitionSpec("core"),) * n_outs
    sharded = jax.jit(
        shard_map(
            _body, mesh=mesh, in_specs=in_specs, out_specs=out_specs,
            check_rep=False,
        ),
        donate_argnums=donate,
        keep_unused=True,
    )

    def run(in_maps):
        concat_in = [
            np.concatenate([np.asarray(m[name]) for m in in_maps], axis=0)
            for name in in_names
        ]
        concat_zeros = [
            np.zeros((NCORES * z.shape[0], *z.shape[1:]), z.dtype)
            for z in zero_outs
        ]
        out_arrs = sharded(*concat_in, *concat_zeros)
        return [
            {
                name: np.asarray(out_arrs[i]).reshape(
                    NCORES, *out_avals[i].shape)[c]
                for i, name in enumerate(out_names)
            }
            for c in range(NCORES)
        ]

    def bench(in_maps, iters=20):
        """Time the sharded call with device-resident inputs."""
        import time
        from jax.sharding import NamedSharding

        concat_in = [
            np.concatenate([np.asarray(m[name]) for m in in_maps], axis=0)
            for name in in_names
        ]
        shard = NamedSharding(mesh, PartitionSpec("core"))
        dev_in = [jax.device_put(x, shard) for x in concat_in]
        zeros = [
            np.zeros((NCORES * z.shape[0], *z.shape[1:]), z.dtype)
            for z in zero_outs
        ]

        def call():
            zs = [jax.device_put(z, shard) for z in zeros]
            outs = sharded(*dev_in, *zs)
            for o in outs:
                o.block_until_ready()

        call()
        ts = []
        for _ in range(iters):
            t0 = time.perf_counter()
            call()
            ts.append(time.perf_counter() - t0)
        return min(ts), sum(ts) / len(ts)

    run.bench = bench
    _cached["runner"] = run
    return run


def kernel(pred_instance_mask, pred_score, cls_out, target_mask):
    run = _get_runner()
    in_maps = make_in_maps(pred_instance_mask, pred_score, cls_out, target_mask)
    outs = [r["out"] for r in run(in_maps)]

    acc = np.zeros((N, K), dtype=np.float64)
    aux = np.zeros((N, 3), dtype=np.float64)
    ps2 = 0.0
    for o in outs:
        o = o.astype(np.float64)
        for q in range(NQUAD):
            blk = o[32 * q:32 * q + N, 0:MCOL]
            duals = blk[:, 0:ND]
            hi = np.floor((duals + 0.5) / SCALE)
            lo = duals - hi * SCALE
            acc[:, 0::2] += lo
            acc[:, 1::2] += hi
            aux += blk[:, ND:ND + 3]
        ps2 += o[:, 40:40 + NCHUNK].sum()

    return _host_finish(acc, aux, ps2)


def _host_finish(inter, aux, ps2):
    st = inter.sum(axis=1)            # [N] target marginals
    sp = inter.sum(axis=0)            # [K] pred marginals
    sum_t = HWPIX - st[0]             # count(target > 0)
    sum_p = aux[:, 0].sum()           # sum(cls_out)
    sum_logp = aux[:, 1].sum()
    inter_cls = sum_p - aux[0, 0]     # sum over target>0 of cls_out
    bce_sum = (sum_logp - aux[0, 1]) + aux[0, 2]

    mse = ps2 / HWPIX
    bce_cls = -bce_sum / HWPIX
    dice_cls = 1.0 - (2.0 * inter_cls + SMOOTH) / (sum_p + sum_t + SMOOTH)

    union = st[:, None] + sp[None, :]
    bce_pair = 100.0 * (union - 2.0 * inter) / HWPIX
    dice_pair = 1.0 - (2.0 * inter + SMOOTH) / (union + SMOOTH)
    pair = bce_pair + dice_pair
    res = mse + bce_cls + dice_cls + pair.min(axis=1).sum()
    return np.float32(res / float(N))
